# revision 1
# baseline (speedup 1.0000x reference)
"""Multi-head attention (B=4, S=2048, D=1024, H=16, DH=64) on 8 TRN2 cores.

Sharding: core c -> (batch b = c//2, head-group g = c%2 of 8 heads).
Each core computes its batch's attention for its 8 heads plus the partial
W_O projection; the host sums the two partial outputs per batch (the
"all-reduce after W_O" done at unshard time).

Device kernel (per core):
  inputs:  xT (D,S) = x[b].T, wq/wk/wv (D, 512) head-major col slices,
           wo (512, D) row slice
  - QT/KT: per head-pair packed (128, S) = (Wq_pair^T x^T), f32r matmuls
  - V: natural (S-chunk, head*65 cols) bf16 with a ones column per head so
    attnV's extra output row = softmax denominator
  - scoresT (s_k, s_q) per head = KT-slice^T x QT-slice (f32r), exp on
    ScalarE -> bf16 P tiles (no max subtraction: scores ~ N(0,1), fp32-safe)
  - O_aug^T (65, s_q) = V_aug^T @ P^T (bf16); row 64 = denom; normalize via
    reciprocal + partition-broadcast multiply
  - out partial (S, D) accumulated over heads via K=64 f32r matmuls with Wo

All DMA-written SBUF tiles are single-assignment (no slot reuse): DMA
descriptors only support one sync-wait command, so input DMAs may only
carry their queue-FIFO wait.
"""

import sys

if "/opt/trn_rl_repo" not in sys.path:
    sys.path.insert(0, "/opt/trn_rl_repo")

import numpy as np

import concourse.bass as bass
import concourse.tile as tile
from concourse import bacc
from concourse import mybir
from concourse import bass_utils

B, S, D, H, DH = 4, 2048, 1024, 16, 64
HL = 8              # heads per core
NCORES = 8
F32 = mybir.dt.float32
F32R = mybir.dt.float32r
BF16 = mybir.dt.bfloat16
EXP = mybir.ActivationFunctionType.Exp

NDC = D // 128      # 8 d-chunks of 128
NKC = S // 128      # 16 s_k chunks of 128
NSQ = S // 512      # 4 s_q chunks of 512


def _kernel_body(tc):
    nc = tc.nc
    xT = nc.dram_tensor("xT", (D, S), F32R, kind="ExternalInput").ap()
    wq = nc.dram_tensor("wq", (D, HL * DH), F32R, kind="ExternalInput").ap()
    wk = nc.dram_tensor("wk", (D, HL * DH), F32R, kind="ExternalInput").ap()
    wv = nc.dram_tensor("wv", (D, HL * DH), F32R, kind="ExternalInput").ap()
    wo = nc.dram_tensor("wo", (HL * DH, D), F32R, kind="ExternalInput").ap()
    out = nc.dram_tensor("out", (S, D), F32, kind="ExternalOutput").ap()

    with tc.tile_pool(name="persist", bufs=1) as persist:
        # Q^T / K^T packed per head pair: rows 0-63 head 2p, 64-127 head 2p+1
        qt = [persist.tile([128, S], F32R, name=f"qt{p}", tag=f"qt{p}") for p in range(4)]
        kt = [persist.tile([128, S], F32R, name=f"kt{p}", tag=f"kt{p}") for p in range(4)]
        # V natural bf16, 65 cols per head (64 V + 1 ones)
        vv = [persist.tile([128, HL * 65], BF16, name=f"v{sc}", tag=f"v{sc}") for sc in range(NKC)]

        # ---------------- Phase A: projections ----------------
        with tc.tile_pool(name="xtp", bufs=8) as xtp, \
             tc.tile_pool(name="wp", bufs=8) as wp, \
             tc.tile_pool(name="psA", bufs=6, space="PSUM") as psA:
            qs = [nc.sync, nc.scalar, nc.gpsimd]
            xt, wqt, wkt, wvt = [], [], [], []
            for dc in range(NDC):
                sl = slice(dc * 128, (dc + 1) * 128)
                t = xtp.tile([128, S], F32R, name=f"xt{dc}", tag="xt")
                for xc in range(4):
                    qs[(dc + xc) % 3].dma_start(out=t[:, xc * 512:(xc + 1) * 512],
                                                in_=xT[sl, xc * 512:(xc + 1) * 512])
                xt.append(t)
                a = wp.tile([128, HL * DH], F32R, name=f"wq{dc}", tag="wq")
                qs[(dc + 1) % 3].dma_start(out=a, in_=wq[sl, :])
                wqt.append(a)
                b_ = wp.tile([128, HL * DH], F32R, name=f"wk{dc}", tag="wk")
                qs[(dc + 2) % 3].dma_start(out=b_, in_=wk[sl, :])
                wkt.append(b_)
                c_ = wp.tile([128, HL * DH], F32R, name=f"wv{dc}", tag="wv")
                qs[dc % 3].dma_start(out=c_, in_=wv[sl, :])
                wvt.append(c_)
            for sc in range(NKC):
                nc.vector.memset(vv[sc], 1.0)

            # Q^T, K^T per head pair
            for p in range(4):
                csl = slice(p * 128, (p + 1) * 128)
                for sq in range(NSQ):
                    ssl = slice(sq * 512, (sq + 1) * 512)
                    ps = psA.tile([128, 512], F32, name=f"psq_{p}_{sq}", tag="ps")
                    for dc in range(NDC):
                        nc.tensor.matmul(ps, wqt[dc][:, csl], xt[dc][:, ssl],
                                         start=(dc == 0), stop=(dc == NDC - 1))
                    nc.vector.tensor_copy(qt[p][:, ssl], ps)
                    ps2 = psA.tile([128, 512], F32, name=f"psk_{p}_{sq}", tag="ps")
                    for dc in range(NDC):
                        nc.tensor.matmul(ps2, wkt[dc][:, csl], xt[dc][:, ssl],
                                         start=(dc == 0), stop=(dc == NDC - 1))
                    nc.vector.tensor_copy(kt[p][:, ssl], ps2)

            # V natural, all 8 heads at once (N=512)
            for sc in range(NKC):
                ps = psA.tile([128, 512], F32, name=f"psv_{sc}", tag="psv", bufs=2)
                for dc in range(NDC):
                    nc.tensor.matmul(ps, xt[dc][:, sc * 128:(sc + 1) * 128], wvt[dc],
                                     start=(dc == 0), stop=(dc == NDC - 1))
                # scatter 8 heads' (128,64) blocks into stride-65 slots
                vsrc = ps.rearrange("p (h x) -> p h x", x=64)
                vdst = vv[sc].rearrange("p (h x) -> p h x", x=65)[:, :, 0:64]
                nc.vector.tensor_copy(vdst, vsrc)

        # ---------------- Phase B: attention + fused out-projection ----------------
        with tc.tile_pool(name="wop", bufs=1) as wop, \
             tc.tile_pool(name="ptp", bufs=22) as ptp, \
             tc.tile_pool(name="otp", bufs=1) as otp, \
             tc.tile_pool(name="rrp", bufs=4) as rrp, \
             tc.tile_pool(name="brp", bufs=4) as brp, \
             tc.tile_pool(name="stg", bufs=5) as stg, \
             tc.tile_pool(name="psS", bufs=2, space="PSUM") as psS, \
             tc.tile_pool(name="psO", bufs=2, space="PSUM") as psO, \
             tc.tile_pool(name="psF", bufs=2, space="PSUM") as psF:

            # Wo per head pair (128 rows = two heads' dh) for K=128 out-proj
            wo_t = [wop.tile([128, D], F32R, name=f"wo{p}", tag=f"wo{p}") for p in range(4)]
            for p in range(4):
                nc.gpsimd.dma_start(out=wo_t[p], in_=wo[p * 128:(p + 1) * 128, :])

            # s_q processed in chunks of 1024 so exp runs on (128,1024) tiles
            otpairs = {}

            def process_head(q2, h):
                # odd head of each pair first: its O^T needs an extra
                # SBUF->SBUF DMA hop, which then overlaps the even head's
                # compute; the last head per pair writes otpair directly
                otpair = otpairs[q2]
                p, rh = h // 2, h % 2
                rsl = slice(rh * 64, (rh + 1) * 64)
                pts = []
                for kc in range(NKC):
                    ps = psS.tile([128, 1024], F32, name=f"pss_{q2}_{h}_{kc}", tag="pss")
                    for half in range(2):
                        nc.tensor.matmul(ps[:, half * 512:(half + 1) * 512],
                                         kt[p][rsl, kc * 128:(kc + 1) * 128],
                                         qt[p][rsl, q2 * 1024 + half * 512: q2 * 1024 + (half + 1) * 512],
                                         start=True, stop=True)
                    pe = ptp.tile([128, 1024], BF16, name=f"pt_{q2}_{h}_{kc}", tag="pt")
                    nc.scalar.activation(pe, ps, EXP, scale=0.125)
                    pts.append(pe)
                osct = None
                if rh == 1:
                    osct = stg.tile([64, 1024], F32R, name=f"os_{q2}_{h}", tag="os", bufs=2)
                for half in range(2):
                    hsl = slice(half * 512, (half + 1) * 512)
                    po = psO.tile([65, 512], F32, name=f"pso_{q2}_{h}_{half}", tag="pso")
                    for kc in range(NKC):
                        nc.tensor.matmul(po, vv[kc][:, h * 65:(h + 1) * 65], pts[kc][:, hsl],
                                         start=(kc == 0), stop=(kc == NKC - 1))
                    rr = rrp.tile([1, 512], F32, name=f"rr_{q2}_{h}_{half}", tag="rr")
                    nc.vector.reciprocal(rr, po[64:65, :])
                    br = brp.tile([64, 512], F32, name=f"br_{q2}_{h}_{half}", tag="br")
                    nc.gpsimd.partition_broadcast(br, rr)
                    if rh == 0:
                        nc.vector.tensor_mul(otpair[p][0:64, hsl], po[0:64, :], br)
                    else:
                        nc.vector.tensor_mul(osct[:, hsl], po[0:64, :], br)
                        # move this half up to partitions 64-127 right away
                        # (gpsimd queue: keep it off the store queue)
                        nc.gpsimd.dma_start(out=otpair[p][64:128, hsl], in_=osct[:, hsl])

            def outproj_group(q2, q16, dcol):
                # K=128 per pair-stacked O^T tile
                otpair = otpairs[q2]
                pf = psF.tile([128, 512], F32, name=f"psf_{q2}_{q16}_{dcol}", tag="psf")
                for p in range(4):
                    nc.tensor.matmul(pf, otpair[p][:, q16 * 128:(q16 + 1) * 128],
                                     wo_t[p][:, dcol * 512:(dcol + 1) * 512],
                                     start=(p == 0), stop=(p == 3))
                st = stg.tile([128, 512], F32, name=f"st_{q2}_{q16}_{dcol}", tag="st")
                nc.vector.tensor_copy(st, pf)
                nc.sync.dma_start(
                    out=out[q2 * 1024 + q16 * 128: q2 * 1024 + (q16 + 1) * 128,
                            dcol * 512:(dcol + 1) * 512],
                    in_=st)

            ORDER = [1, 0, 3, 2, 5, 4, 7, 6]
            GROUPS = [(q16, dcol) for q16 in range(8) for dcol in range(2)]
            for q2 in range(2):
                # O^T pair-stacked: head 2p on partitions 0-63, head 2p+1 on
                # 64-127 (odd head moved up via SBUF->SBUF DMA)
                otpairs[q2] = [otp.tile([128, 1024], F32R, name=f"otp_{q2}_{p}",
                                        tag=f"otp{p}", bufs=2) for p in range(4)]
                for i, h in enumerate(ORDER):
                    process_head(q2, h)
                    if q2 == 1:
                        # interleave q2=0's out-projection as PE filler while
                        # ACT paces q2=1's exps
                        for g in GROUPS[2 * i: 2 * i + 2]:
                            outproj_group(0, *g)
            for g in GROUPS:
                outproj_group(1, *g)


_NC_CACHE = None


def _get_nc():
    global _NC_CACHE
    if _NC_CACHE is None:
        nc = bacc.Bacc("TRN2", target_bir_lowering=False, debug=False)
        with tile.TileContext(nc) as tc:
            _kernel_body(tc)
        nc.compile()
        _NC_CACHE = nc
    return _NC_CACHE


def _shard_inputs(x, Wq, Wk, Wv, Wo):
    in_maps = []
    for c in range(NCORES):
        b, g = c // 2, c % 2
        xT = np.ascontiguousarray(x[b].T)
        sl = slice(HL * g, HL * (g + 1))
        wq_s = np.ascontiguousarray(Wq[sl].transpose(1, 0, 2).reshape(D, HL * DH))
        wk_s = np.ascontiguousarray(Wk[sl].transpose(1, 0, 2).reshape(D, HL * DH))
        wv_s = np.ascontiguousarray(Wv[sl].transpose(1, 0, 2).reshape(D, HL * DH))
        wo_s = np.ascontiguousarray(Wo[HL * DH * g: HL * DH * (g + 1), :])
        in_maps.append({"xT": xT, "wq": wq_s, "wk": wk_s, "wv": wv_s, "wo": wo_s})
    return in_maps


def kernel(**inputs):
    x = np.asarray(inputs["x"], dtype=np.float32)
    Wq = np.asarray(inputs["Wq"], dtype=np.float32)
    Wk = np.asarray(inputs["Wk"], dtype=np.float32)
    Wv = np.asarray(inputs["Wv"], dtype=np.float32)
    Wo = np.asarray(inputs["Wo"], dtype=np.float32)

    nc = _get_nc()
    in_maps = _shard_inputs(x, Wq, Wk, Wv, Wo)
    res = None
    for attempt in range(3):
        try:
            res = bass_utils.run_bass_kernel_spmd(nc, in_maps, core_ids=list(range(NCORES)))
            break
        except Exception:
            # transient axon/NRT device errors recover on retry
            if attempt == 2:
                raise
            import time
            time.sleep(20)
    outs = [res.results[c]["out"] for c in range(NCORES)]
    full = np.stack([outs[2 * b] + outs[2 * b + 1] for b in range(B)], axis=0)
    return full.astype(np.float32)



# revision 5
# speedup vs baseline: 1.1112x; 1.1112x over previous
"""Multi-head attention (B=4, S=2048, D=1024, H=16, DH=64) on 8 TRN2 cores.

Sharding: core c -> (batch b = c//2, head-group g = c%2 of 8 heads).
Each core computes its batch's attention for its 8 heads plus the partial
W_O projection; the host sums the two partial outputs per batch.

Device kernel (per core), software-pipelined so ACT (exp) never starves:
  - all operands downcast to bf16 on device (f32 DMA staging ring -> bf16
    tiles); PSUM accumulation stays f32, measured rel err ~5e-3.
  - Q^T/K^T per head pair packed [128, S] bf16; V natural [sk, 65*8] bf16
    with a ones column per head (softmax denominator via the attn.V matmul).
  - scores^T per (head, 1024-col sq window) as 16 PSUM tiles [128,1024],
    exp on ACT -> bf16 P tiles (no max subtraction; scores ~ N(0,1)).
  - attn.V in natural-O orientation: per sq-chunk psO[128,65] accumulates
    16 kc matmuls (N=65) -- half the PE cycles of the O^T orientation.
  - normalize: DVE reciprocal of the ones column + broadcast multiply into
    per-pair [128,128] staging, then DMA XBAR transpose into O^T tiles.
  - out projection K=512 (all 4 pairs) into PSUM, Pool copy, DMA to DRAM.
  Head loop interleaves window pairs (h_even,w0),(h_odd,w0),(h_even,w1),
  (h_odd,w1) per round; projections for pair p+1 and V for the next two
  heads are PE filler inside round p's exp windows.
"""

import itertools
import sys

if "/opt/trn_rl_repo" not in sys.path:
    sys.path.insert(0, "/opt/trn_rl_repo")

import numpy as np

import concourse.bass as bass
import concourse.tile as tile
from concourse import bacc
from concourse import mybir
from concourse import bass_utils

B, S, D, H, DH = 4, 2048, 1024, 16, 64
HL = 8              # heads per core
NCORES = 8
F32 = mybir.dt.float32
BF16 = mybir.dt.bfloat16
EXP = mybir.ActivationFunctionType.Exp

NDC = D // 128      # 8 d-chunks of 128
NKC = S // 128      # 16 s_k chunks of 128
NW = 2              # sq windows of 1024
NC8 = 8             # 128-wide sq chunks per window


def _kernel_body(tc):
    nc = tc.nc
    xT = nc.dram_tensor("xT", (D, S), F32, kind="ExternalInput").ap()
    wq = nc.dram_tensor("wq", (D, HL * DH), F32, kind="ExternalInput").ap()
    wk = nc.dram_tensor("wk", (D, HL * DH), F32, kind="ExternalInput").ap()
    wv = nc.dram_tensor("wv", (D, HL * DH), F32, kind="ExternalInput").ap()
    wo = nc.dram_tensor("wo", (HL * DH, D), F32, kind="ExternalInput").ap()
    out = nc.dram_tensor("out", (S, D), F32, kind="ExternalOutput").ap()

    cnt = itertools.count()
    conv_engines = itertools.cycle([nc.vector, nc.gpsimd])

    with tc.tile_pool(name="persist", bufs=1) as persist:
        qt = [persist.tile([128, S], BF16, name=f"qt{p}", tag=f"qt{p}") for p in range(4)]
        kt = [persist.tile([128, S], BF16, name=f"kt{p}", tag=f"kt{p}") for p in range(4)]
        vv = [persist.tile([128, HL * 65], BF16, name=f"v{sc}", tag=f"v{sc}") for sc in range(NKC)]
        ot = [persist.tile([128, S], BF16, name=f"ot{p}", tag=f"ot{p}") for p in range(4)]
        wqb = [persist.tile([128, HL * DH], BF16, name=f"wqb{dc}", tag=f"wqb{dc}") for dc in range(NDC)]
        wkb = [persist.tile([128, HL * DH], BF16, name=f"wkb{dc}", tag=f"wkb{dc}") for dc in range(NDC)]
        wvb = [persist.tile([128, HL * DH], BF16, name=f"wvb{dc}", tag=f"wvb{dc}") for dc in range(NDC)]

        with tc.tile_pool(name="stage", bufs=8) as stage, \
             tc.tile_pool(name="ptsp", bufs=26) as ptsp, \
             tc.tile_pool(name="onatp", bufs=18) as onatp, \
             tc.tile_pool(name="rrp", bufs=8) as rrp, \
             tc.tile_pool(name="stp", bufs=3) as stp, \
             tc.tile_pool(name="psS", bufs=2, space="PSUM") as psS, \
             tc.tile_pool(name="psO", bufs=2, space="PSUM") as psO, \
             tc.tile_pool(name="ppp", bufs=2, space="PSUM") as ppp:

            # bf16 x^T tiles live until the last projection (round-2 filler).
            # Pools release in LIFO order, so this one sits on top of the
            # stack and is swapped for the wo bf16 tiles at round 3.
            xbp = tc.alloc_tile_pool(name="xbp", bufs=1)
            xb = [xbp.tile([128, S], BF16, name=f"xb{dc}", tag=f"xb{dc}")
                  for dc in range(NDC)]

            wob = [None] * 4
            pts_map = {}
            onat_map = {}

            def load(dram_slice, dst_slice, shape):
                n = next(cnt)
                t = stage.tile(list(shape), F32, name=f"sg{n}", tag="sg")
                nc.sync.dma_start(out=t, in_=dram_slice)
                next(conv_engines).tensor_copy(dst_slice, t)

            def load_x_block(cb):
                csl = slice(cb * 512, (cb + 1) * 512)
                for dc in range(NDC):
                    load(xT[dc * 128:(dc + 1) * 128, csl], xb[dc][:, csl], (128, 512))

            def load_w_cols(dram_w, dst_list, c0, c1):
                for dc in range(NDC):
                    load(dram_w[dc * 128:(dc + 1) * 128, c0:c1],
                         dst_list[dc][:, c0:c1], (128, c1 - c0))

            def qk_proj(which, p, s4):
                wsrc = wqb if which == "q" else wkb
                dst = (qt if which == "q" else kt)[p]
                ssl = slice(s4 * 512, (s4 + 1) * 512)
                ps = ppp.tile([128, 512], F32, name=f"pp_{which}{p}_{s4}", tag="pp")
                for dc in range(NDC):
                    nc.tensor.matmul(ps, wsrc[dc][:, p * 128:(p + 1) * 128],
                                     xb[dc][:, ssl], start=(dc == 0), stop=(dc == NDC - 1))
                nc.vector.tensor_copy(dst[:, ssl], ps)

            def v_proj(h):
                for sc in range(NKC):
                    ps = ppp.tile([128, DH], F32, name=f"pv_{h}_{sc}", tag="pp")
                    for dc in range(NDC):
                        nc.tensor.matmul(ps, xb[dc][:, sc * 128:(sc + 1) * 128],
                                         wvb[dc][:, h * DH:(h + 1) * DH],
                                         start=(dc == 0), stop=(dc == NDC - 1))
                    nc.vector.tensor_copy(vv[sc][:, h * 65:h * 65 + 64], ps)

            def scores(h, w, kcs):
                p, rh = h // 2, h % 2
                rsl = slice(rh * 64, rh * 64 + 64)
                lst = pts_map.setdefault((h, w), [])
                for kc in kcs:
                    ps = psS.tile([128, 1024], F32, name=f"ps_{h}_{w}_{kc}", tag="ps")
                    for half in range(2):
                        q0 = w * 1024 + half * 512
                        nc.tensor.matmul(ps[:, half * 512:(half + 1) * 512],
                                         kt[p][rsl, kc * 128:(kc + 1) * 128],
                                         qt[p][rsl, q0:q0 + 512],
                                         start=True, stop=True)
                    pt = ptsp.tile([128, 1024], BF16, name=f"pt_{h}_{w}_{kc}", tag="pt")
                    nc.scalar.activation(pt, ps, EXP, scale=0.125)
                    lst.append(pt)

            def outproj_chunk(w, c):
                q16 = w * 8 + c
                qsl = slice(q16 * 128, (q16 + 1) * 128)
                for dcol in range(2):
                    pf = ppp.tile([128, 512], F32, name=f"pf_{q16}_{dcol}", tag="pp")
                    for p in range(4):
                        nc.tensor.matmul(pf, ot[p][:, qsl],
                                         wob[p][:, dcol * 512:(dcol + 1) * 512],
                                         start=(p == 0), stop=(p == 3))
                    st = stp.tile([128, 512], F32, name=f"st_{q16}_{dcol}", tag="st")
                    nc.vector.tensor_copy(st, pf)
                    nc.sync.dma_start(out=out[qsl, dcol * 512:(dcol + 1) * 512], in_=st)

            def attn_v(h, w, fuse_outproj=False):
                p, rh = h // 2, h % 2
                P = pts_map.pop((h, w))
                for c in range(NC8):
                    po = psO.tile([128, 65], F32, name=f"po_{h}_{w}_{c}", tag="po")
                    for kc in range(NKC):
                        nc.tensor.matmul(po, P[kc][:, c * 128:(c + 1) * 128],
                                         vv[kc][:, h * 65:(h + 1) * 65],
                                         start=(kc == 0), stop=(kc == NKC - 1))
                    rr = rrp.tile([128, 1], F32, name=f"rr_{h}_{w}_{c}", tag="rr")
                    nc.vector.reciprocal(rr, po[:, 64:65])
                    if rh == 0:
                        on = onatp.tile([128, 128], BF16, name=f"on_{w}_{c}_{p}", tag="on")
                        onat_map[(w, c)] = on
                    else:
                        on = onat_map.pop((w, c))
                    nc.vector.tensor_mul(on[:, rh * 64:(rh + 1) * 64], po[:, 0:64],
                                         rr[:, 0:1].broadcast_to((128, 64)))
                    if rh == 1:
                        nc.sync.dma_start_transpose(
                            out=ot[p][:, w * 1024 + c * 128: w * 1024 + (c + 1) * 128],
                            in_=on)
                        if fuse_outproj:
                            outproj_chunk(w, c)

            # ---------------- fill ----------------
            for sc in range(NKC):
                nc.gpsimd.memset(vv[sc], 1.0)
            load_x_block(0)
            load_x_block(1)
            load_w_cols(wq, wqb, 0, 128)   # pair 0
            load_w_cols(wk, wkb, 0, 128)
            load_x_block(2)
            load_x_block(3)
            load_w_cols(wv, wvb, 0, 512)

            qk_proj("q", 0, 0)
            qk_proj("q", 0, 1)
            for s4 in range(4):
                qk_proj("k", 0, s4)
                scores(0, 0, range(4 * s4, 4 * s4 + 4))
            qk_proj("q", 0, 2)
            qk_proj("q", 0, 3)
            v_proj(0)
            v_proj(1)

            # ---------------- rounds ----------------
            for r in range(4):
                a, b = 2 * r, 2 * r + 1
                if r == 0:
                    load_w_cols(wk, wkb, 128, 512)
                    load_w_cols(wq, wqb, 128, 512)
                if r == 3:
                    # xb space is done with (last projections were round-2
                    # filler); reuse it for the wo bf16 tiles.
                    xbp.release()
                    wobp = tc.alloc_tile_pool(name="wobp", bufs=1)
                    for p in range(4):
                        wob[p] = wobp.tile([128, D], BF16, name=f"wob{p}", tag=f"wob{p}")
                        for half in range(2):
                            load(wo[p * 128:(p + 1) * 128, half * 512:(half + 1) * 512],
                                 wob[p][:, half * 512:(half + 1) * 512], (128, 512))
                if r > 0:
                    attn_v(2 * r - 1, 1)
                    scores(a, 0, range(NKC))
                scores(b, 0, range(NKC))
                attn_v(a, 0)
                scores(a, 1, range(NKC))
                attn_v(b, 0)
                scores(b, 1, range(NKC))
                attn_v(a, 1)
                if r < 3:
                    v_proj(2 * r + 2)
                    v_proj(2 * r + 3)
                    for s4 in range(4):
                        qk_proj("q", r + 1, s4)
                        qk_proj("k", r + 1, s4)
                else:
                    for c in range(NC8):
                        outproj_chunk(0, c)

            # ---------------- drain ----------------
            attn_v(7, 1, fuse_outproj=True)
            wobp.release()


_NC_CACHE = None


def _get_nc():
    global _NC_CACHE
    if _NC_CACHE is None:
        nc = bacc.Bacc("TRN2", target_bir_lowering=False, debug=False)
        with tile.TileContext(nc) as tc:
            _kernel_body(tc)
        nc.compile()
        _NC_CACHE = nc
    return _NC_CACHE


def _shard_inputs(x, Wq, Wk, Wv, Wo):
    in_maps = []
    for c in range(NCORES):
        b, g = c // 2, c % 2
        xT = np.ascontiguousarray(x[b].T)
        sl = slice(HL * g, HL * (g + 1))
        wq_s = np.ascontiguousarray(Wq[sl].transpose(1, 0, 2).reshape(D, HL * DH))
        wk_s = np.ascontiguousarray(Wk[sl].transpose(1, 0, 2).reshape(D, HL * DH))
        wv_s = np.ascontiguousarray(Wv[sl].transpose(1, 0, 2).reshape(D, HL * DH))
        wo_s = np.ascontiguousarray(Wo[HL * DH * g: HL * DH * (g + 1), :])
        in_maps.append({"xT": xT, "wq": wq_s, "wk": wk_s, "wv": wv_s, "wo": wo_s})
    return in_maps


def kernel(**inputs):
    x = np.asarray(inputs["x"], dtype=np.float32)
    Wq = np.asarray(inputs["Wq"], dtype=np.float32)
    Wk = np.asarray(inputs["Wk"], dtype=np.float32)
    Wv = np.asarray(inputs["Wv"], dtype=np.float32)
    Wo = np.asarray(inputs["Wo"], dtype=np.float32)

    nc = _get_nc()
    in_maps = _shard_inputs(x, Wq, Wk, Wv, Wo)
    res = None
    for attempt in range(3):
        try:
            res = bass_utils.run_bass_kernel_spmd(nc, in_maps, core_ids=list(range(NCORES)))
            break
        except Exception:
            # transient axon/NRT device errors recover on retry
            if attempt == 2:
                raise
            import time
            time.sleep(20)
    outs = [res.results[c]["out"] for c in range(NCORES)]
    full = np.stack([outs[2 * b] + outs[2 * b + 1] for b in range(B)], axis=0)
    return full.astype(np.float32)


# revision 28
# speedup vs baseline: 1.1854x; 1.0667x over previous
"""Multi-head attention (B=4, S=2048, D=1024, H=16, DH=64) on 8 TRN2 cores.

Sharding: core c -> (batch b = c//2, head-group g = c%2 of 8 heads).
Each core computes its batch's attention for its 8 heads plus the partial
W_O projection; the host sums the two partial outputs per batch.

Device kernel (per core), software-pipelined so ACT (exp) never starves:
  - all operands downcast to bf16 on device (f32 DMA staging ring -> bf16
    tiles); PSUM accumulation stays f32, measured rel err ~5e-3.
  - Q^T/K^T per head pair packed [128, S] bf16; V natural [sk, 65*8] bf16
    with a ones column per head (softmax denominator via the attn.V matmul).
  - scores^T per (head, 1024-col sq window) as 16 PSUM tiles [128,1024],
    exp on ACT -> bf16 P tiles (no max subtraction; scores ~ N(0,1)).
  - attn.V in natural-O orientation: per sq-chunk psO[128,65] accumulates
    16 kc matmuls (N=65) -- half the PE cycles of the O^T orientation.
  - normalize: DVE reciprocal of the ones column + broadcast multiply into
    per-pair [128,128] staging, then DMA XBAR transpose into O^T tiles.
  - out projection K=512 (all 4 pairs) into PSUM, Pool copy, DMA to DRAM.
  Head loop interleaves window pairs (h_even,w0),(h_odd,w0),(h_even,w1),
  (h_odd,w1) per round; projections for pair p+1 and V for the next two
  heads are PE filler inside round p's exp windows.
"""

import itertools
import sys

if "/opt/trn_rl_repo" not in sys.path:
    sys.path.insert(0, "/opt/trn_rl_repo")

import numpy as np

import concourse.bass as bass
import concourse.tile as tile
from concourse import bacc
from concourse import mybir
from concourse import bass_utils

B, S, D, H, DH = 4, 2048, 1024, 16, 64
HL = 8              # heads per core
NCORES = 8
F32 = mybir.dt.float32
BF16 = mybir.dt.bfloat16
EXP = mybir.ActivationFunctionType.Exp

NDC = D // 128      # 8 d-chunks of 128
NKC = S // 128      # 16 s_k chunks of 128
NW = 2              # sq windows of 1024
NC8 = 8             # 128-wide sq chunks per window


def _kernel_body(tc):
    nc = tc.nc
    xT = nc.dram_tensor("xT", (D, S), F32, kind="ExternalInput").ap()
    wq = nc.dram_tensor("wq", (D, HL * DH), F32, kind="ExternalInput").ap()
    wk = nc.dram_tensor("wk", (D, HL * DH), F32, kind="ExternalInput").ap()
    wv = nc.dram_tensor("wv", (D, HL * DH), F32, kind="ExternalInput").ap()
    wo = nc.dram_tensor("wo", (HL * DH, D), F32, kind="ExternalInput").ap()
    out = nc.dram_tensor("out", (S, D), F32, kind="ExternalOutput").ap()

    cnt = itertools.count()
    conv_engines = itertools.cycle([nc.vector, nc.gpsimd])

    with tc.tile_pool(name="persist", bufs=1) as persist:
        qt = [persist.tile([128, S], BF16, name=f"qt{p}", tag=f"qt{p}") for p in range(4)]
        kt = [persist.tile([128, S], BF16, name=f"kt{p}", tag=f"kt{p}") for p in range(4)]
        vv = [persist.tile([128, HL * 65], BF16, name=f"v{sc}", tag=f"v{sc}") for sc in range(NKC)]
        ot = [persist.tile([128, S], BF16, name=f"ot{p}", tag=f"ot{p}") for p in range(4)]
        wqb = [persist.tile([128, HL * DH], BF16, name=f"wqb{dc}", tag=f"wqb{dc}") for dc in range(NDC)]
        wkb = [persist.tile([128, HL * DH], BF16, name=f"wkb{dc}", tag=f"wkb{dc}") for dc in range(NDC)]
        wvb = [persist.tile([128, HL * DH], BF16, name=f"wvb{dc}", tag=f"wvb{dc}") for dc in range(NDC)]

        with tc.tile_pool(name="stage", bufs=5) as stage, \
             tc.tile_pool(name="ptsp", bufs=26) as ptsp, \
             tc.tile_pool(name="onatp", bufs=18) as onatp, \
             tc.tile_pool(name="rrp", bufs=8) as rrp, \
             tc.tile_pool(name="stp", bufs=3) as stp, \
             tc.tile_pool(name="psS", bufs=2, space="PSUM") as psS, \
             tc.tile_pool(name="psO", bufs=2, space="PSUM") as psO, \
             tc.tile_pool(name="ppp", bufs=2, space="PSUM") as ppp:

            # bf16 x^T tiles live until the last projection (round-2 filler).
            # Pools release in LIFO order, so this one sits on top of the
            # stack and is swapped for the wo bf16 tiles at round 3.
            xbp = tc.alloc_tile_pool(name="xbp", bufs=1)
            xb = [xbp.tile([128, S], BF16, name=f"xb{dc}", tag=f"xb{dc}")
                  for dc in range(NDC)]

            wob = [None] * 4
            pts_map = {}
            onat_map = {}
            ppp_of = {"pp": ppp, "ps": psS}

            def load(dram_slice, dst_slice, shape, eng=None):
                n = next(cnt)
                t = stage.tile(list(shape), F32, name=f"sg{n}", tag="sg")
                nc.sync.dma_start(out=t, in_=dram_slice)
                (eng or next(conv_engines)).tensor_copy(dst_slice, t)

            def load_x_half(xh):
                csl = slice(xh * 1024, (xh + 1) * 1024)
                for dc in range(NDC):
                    load(xT[dc * 128:(dc + 1) * 128, csl], xb[dc][:, csl], (128, 1024))

            def load_w_cols(dram_w, dst_list, c0, c1):
                for dc in range(NDC):
                    load(dram_w[dc * 128:(dc + 1) * 128, c0:c1],
                         dst_list[dc][:, c0:c1], (128, c1 - c0))

            def qk_proj(which, p, s4):
                wsrc = wqb if which == "q" else wkb
                dst = (qt if which == "q" else kt)[p]
                ssl = slice(s4 * 512, (s4 + 1) * 512)
                ps = ppp.tile([128, 512], F32, name=f"pp_{which}{p}_{s4}", tag="pp")
                for dc in range(NDC):
                    nc.tensor.matmul(ps, wsrc[dc][:, p * 128:(p + 1) * 128],
                                     xb[dc][:, ssl], start=(dc == 0), stop=(dc == NDC - 1))
                nc.vector.tensor_copy(dst[:, ssl], ps)

            def v_proj(h):
                for sc in range(NKC):
                    ps = ppp.tile([128, DH], F32, name=f"pv_{h}_{sc}", tag="pp")
                    for dc in range(NDC):
                        nc.tensor.matmul(ps, xb[dc][:, sc * 128:(sc + 1) * 128],
                                         wvb[dc][:, h * DH:(h + 1) * DH],
                                         start=(dc == 0), stop=(dc == NDC - 1))
                    nc.vector.tensor_copy(vv[sc][:, h * 65:h * 65 + 64], ps)

            def scores(h, w, kcs):
                p, rh = h // 2, h % 2
                rsl = slice(rh * 64, rh * 64 + 64)
                dct = pts_map.setdefault((h, w), {})
                for kc in kcs:
                    ps = psS.tile([128, 1024], F32, name=f"ps_{h}_{w}_{kc}", tag="ps")
                    for half in range(2):
                        q0 = w * 1024 + half * 512
                        nc.tensor.matmul(ps[:, half * 512:(half + 1) * 512],
                                         kt[p][rsl, kc * 128:(kc + 1) * 128],
                                         qt[p][rsl, q0:q0 + 512],
                                         start=True, stop=True)
                    pt = ptsp.tile([128, 1024], BF16, name=f"pt_{h}_{w}_{kc}", tag="pt")
                    nc.scalar.activation(pt, ps, EXP, scale=0.125)
                    dct[kc] = pt

            def outproj_chunk(w, c, drain=False):
                # drain mode: ACT (idle after the last exp) does the PSUM->SBUF
                # copies, and psF alternates between the pp and ps PSUM rings
                # for a depth-4 pipeline so the PE never stalls on a copy.
                q16 = w * 8 + c
                qsl = slice(q16 * 128, (q16 + 1) * 128)
                for dcol in range(2):
                    tag = ("ps" if (drain and dcol == 1) else "pp")
                    shape = [128, 1024] if tag == "ps" else [128, 512]
                    pf = ppp_of[tag].tile(shape, F32, name=f"pf_{q16}_{dcol}", tag=tag)
                    pf = pf[:, 0:512]
                    for p in range(4):
                        nc.tensor.matmul(pf, ot[p][:, qsl],
                                         wob[p][:, dcol * 512:(dcol + 1) * 512],
                                         start=(p == 0), stop=(p == 3))
                    st = stp.tile([128, 512], F32, name=f"st_{q16}_{dcol}", tag="st")
                    if drain:
                        nc.scalar.copy(st, pf)
                    else:
                        nc.vector.tensor_copy(st, pf)
                    nc.sync.dma_start(out=out[qsl, dcol * 512:(dcol + 1) * 512], in_=st)

            def pe_transpose(p, w, c, on, eng):
                # XBAR DMA transpose RTT is ~3.5us through SP/HWDGE/DGE/sem;
                # a PE transpose (128 cycles) + engine copy lands in ~1us and
                # keeps the SP DMA queue free for the output writes.
                tp = psO.tile([128, 128], BF16, name=f"tp_{w}_{c}", tag="po")
                nc.tensor.transpose(tp, on, ident)
                dst = ot[p][:, w * 1024 + c * 128: w * 1024 + (c + 1) * 128]
                if eng is nc.scalar:
                    eng.copy(dst, tp)
                else:
                    eng.tensor_copy(dst, tp)

            def attn_v_chunk(h, w, c, P, transpose=True):
                p, rh = h // 2, h % 2
                po = psO.tile([128, 65], F32, name=f"po_{h}_{w}_{c}", tag="po")
                for kc in range(NKC):
                    nc.tensor.matmul(po, P[kc][:, c * 128:(c + 1) * 128],
                                     vv[kc][:, h * 65:(h + 1) * 65],
                                     start=(kc == 0), stop=(kc == NKC - 1))
                rr = rrp.tile([128, 1], F32, name=f"rr_{h}_{w}_{c}", tag="rr")
                nc.vector.reciprocal(rr, po[:, 64:65])
                if rh == 0:
                    on = onatp.tile([128, 128], BF16, name=f"on_{w}_{c}_{p}", tag="on")
                    onat_map[(w, c)] = on
                else:
                    on = onat_map.pop((w, c))
                nc.vector.tensor_mul(on[:, rh * 64:(rh + 1) * 64], po[:, 0:64],
                                     rr[:, 0:1].broadcast_to((128, 64)))
                if rh == 1 and transpose:
                    pe_transpose(p, w, c, on, nc.vector)
                return on

            def attn_v(h, w):
                P = pts_map.pop((h, w))
                for c in range(NC8):
                    attn_v_chunk(h, w, c, P)

            def attn_v_drain(h, w):
                P = pts_map.pop((h, w))
                p = h // 2
                pend = []
                for c in range(NC8):
                    pend.append((c, attn_v_chunk(h, w, c, P, transpose=False)))
                    if c >= 1:
                        cc, oo = pend.pop(0)
                        pe_transpose(p, w, cc, oo, nc.scalar)
                    if c >= 3:
                        outproj_chunk(w, c - 3, drain=True)
                cc, oo = pend.pop(0)
                pe_transpose(p, w, cc, oo, nc.scalar)
                for c in range(NC8 - 3, NC8):
                    outproj_chunk(w, c, drain=True)

            # ---------------- fill ----------------
            ident = persist.tile([128, 128], BF16, name="ident", tag="ident")
            onesq = persist.tile([128, 128], BF16, name="onesq", tag="onesq")
            nc.gpsimd.memset(onesq, 1.0)
            # iota(p, j) = p - j; keep where == 0 -> identity matrix
            nc.gpsimd.affine_select(ident, onesq, pattern=[[-1, 128]],
                                    compare_op=mybir.AluOpType.is_equal,
                                    fill=0.0, base=0, channel_multiplier=1)
            for sc in range(NKC):
                nc.gpsimd.memset(vv[sc], 1.0)
            load_x_half(0)
            load_w_cols(wq, wqb, 0, 128)   # pair 0
            load_w_cols(wk, wkb, 0, 128)
            load_x_half(1)
            load_w_cols(wv, wvb, 0, 512)

            # everything below "k2" needs the second x half; h0/h1 scores on
            # the first 8 kc chunks keep ACT fed until it lands.
            qk_proj("q", 0, 0)
            qk_proj("q", 0, 1)
            qk_proj("k", 0, 0)
            scores(0, 0, range(0, 4))
            qk_proj("k", 0, 1)
            scores(0, 0, range(4, 8))
            scores(1, 0, range(0, 8))
            qk_proj("k", 0, 2)
            qk_proj("k", 0, 3)
            scores(0, 0, range(8, 16))
            scores(1, 0, range(8, 16))
            qk_proj("q", 0, 2)
            qk_proj("q", 0, 3)
            v_proj(0)

            # PE filler (projections for later heads) per (round, window),
            # balanced so no window's filler exceeds the ~10us of PE slack
            # inside one 16.6us exp window, and so xb's last reader is in
            # round 2's second window (freeing its space for the wo tiles
            # well before the out-projection needs them).
            def F(*items):
                return list(items)
            Q, K, V = "q", "k", None
            fillers = {
                (0, 1): F(("v", 1), (Q, 1, 0), (Q, 1, 1)),
                (0, 2): F(("v", 2), (K, 1, 0), (K, 1, 1)),
                (0, 3): F(("v", 3), (Q, 1, 2), (Q, 1, 3), (K, 1, 2), (K, 1, 3)),
                (1, 1): F(("v", 4), (Q, 2, 0), (Q, 2, 1)),
                (1, 2): F(("v", 5), (K, 2, 0), (K, 2, 1)),
                (1, 3): F((K, 2, 2), (K, 2, 3), (Q, 2, 2), (Q, 2, 3),
                          (Q, 3, 0), (Q, 3, 1)),
                (2, 1): F(("v", 6), (Q, 3, 2), (Q, 3, 3), (K, 3, 0), (K, 3, 1)),
                (2, 2): F(("v", 7), (K, 3, 2), (K, 3, 3)),
            }

            def run_fillers(r, wslot):
                for it in fillers.get((r, wslot), []):
                    if it[0] == "v":
                        v_proj(it[1])
                    else:
                        qk_proj(it[0], it[1], it[2])

            # ---------------- rounds ----------------
            for r in range(4):
                a, b = 2 * r, 2 * r + 1
                if r == 0:
                    load_w_cols(wk, wkb, 128, 512)
                    load_w_cols(wq, wqb, 128, 512)
                if r == 3:
                    # xb space is free by now (last reader was round-2 W2
                    # filler); reuse it for the wo bf16 tiles. Conversions go
                    # on gpsimd so the DVE stream is not blocked behind them.
                    xbp.release()
                    wobp = tc.alloc_tile_pool(name="wobp", bufs=1)
                    for p in range(4):
                        wob[p] = wobp.tile([128, D], BF16, name=f"wob{p}", tag=f"wob{p}")
                        for half in range(2):
                            load(wo[p * 128:(p + 1) * 128, half * 512:(half + 1) * 512],
                                 wob[p][:, half * 512:(half + 1) * 512], (128, 512),
                                 eng=nc.gpsimd)
                if r > 0:
                    scores(a, 0, range(NKC))
                    attn_v(b - 2, 1)
                    scores(b, 0, range(NKC))
                attn_v(a, 0)
                run_fillers(r, 1)
                scores(a, 1, range(NKC))
                attn_v(b, 0)
                run_fillers(r, 2)
                scores(b, 1, range(NKC))
                if r == 3:
                    # outproj(w0) split around attn_v(6,1): the first half
                    # fills the PE while e(6,1) finishes; attn_v(6,1) then
                    # frees the P ring for e(7,1) before the second half.
                    for c in range(4):
                        outproj_chunk(0, c)
                attn_v(a, 1)
                if r == 3:
                    for c in range(4, NC8):
                        outproj_chunk(0, c)
                run_fillers(r, 3)

            # ---------------- drain ----------------
            attn_v_drain(7, 1)
            wobp.release()


_NC_CACHE = None


def _get_nc():
    global _NC_CACHE
    if _NC_CACHE is None:
        nc = bacc.Bacc("TRN2", target_bir_lowering=False, debug=False)
        with tile.TileContext(nc) as tc:
            _kernel_body(tc)
        nc.compile()
        _NC_CACHE = nc
    return _NC_CACHE


def _shard_inputs(x, Wq, Wk, Wv, Wo):
    in_maps = []
    for c in range(NCORES):
        b, g = c // 2, c % 2
        xT = np.ascontiguousarray(x[b].T)
        sl = slice(HL * g, HL * (g + 1))
        wq_s = np.ascontiguousarray(Wq[sl].transpose(1, 0, 2).reshape(D, HL * DH))
        wk_s = np.ascontiguousarray(Wk[sl].transpose(1, 0, 2).reshape(D, HL * DH))
        wv_s = np.ascontiguousarray(Wv[sl].transpose(1, 0, 2).reshape(D, HL * DH))
        wo_s = np.ascontiguousarray(Wo[HL * DH * g: HL * DH * (g + 1), :])
        in_maps.append({"xT": xT, "wq": wq_s, "wk": wk_s, "wv": wv_s, "wo": wo_s})
    return in_maps


def kernel(**inputs):
    x = np.asarray(inputs["x"], dtype=np.float32)
    Wq = np.asarray(inputs["Wq"], dtype=np.float32)
    Wk = np.asarray(inputs["Wk"], dtype=np.float32)
    Wv = np.asarray(inputs["Wv"], dtype=np.float32)
    Wo = np.asarray(inputs["Wo"], dtype=np.float32)

    nc = _get_nc()
    in_maps = _shard_inputs(x, Wq, Wk, Wv, Wo)
    res = None
    for attempt in range(3):
        try:
            res = bass_utils.run_bass_kernel_spmd(nc, in_maps, core_ids=list(range(NCORES)))
            break
        except Exception:
            # transient axon/NRT device errors recover on retry
            if attempt == 2:
                raise
            import time
            time.sleep(20)
    outs = [res.results[c]["out"] for c in range(NCORES)]
    full = np.stack([outs[2 * b] + outs[2 * b + 1] for b in range(B)], axis=0)
    return full.astype(np.float32)


# revision 43
# speedup vs baseline: 1.1869x; 1.0013x over previous
"""Multi-head attention (B=4, S=2048, D=1024, H=16, DH=64) on 8 TRN2 cores.

Sharding: core c -> (batch b = c//2, head-group g = c%2 of 8 heads).
Each core computes its batch's attention for its 8 heads plus the partial
W_O projection; the host sums the two partial outputs per batch.

Device kernel (per core), software-pipelined so ACT (exp) never starves:
  - all operands downcast to bf16 on device (f32 DMA staging ring -> bf16
    tiles); PSUM accumulation stays f32, measured rel err ~5e-3.
  - Q^T/K^T per head pair packed [128, S] bf16; V natural [sk, 65*8] bf16
    with a ones column per head (softmax denominator via the attn.V matmul).
  - scores^T per (head, 1024-col sq window) as 16 PSUM tiles [128,1024],
    exp on ACT -> bf16 P tiles (no max subtraction; scores ~ N(0,1)).
  - attn.V in natural-O orientation: per sq-chunk psO[128,65] accumulates
    16 kc matmuls (N=65) -- half the PE cycles of the O^T orientation.
  - normalize: DVE reciprocal of the ones column + broadcast multiply into
    per-pair [128,128] staging, then DMA XBAR transpose into O^T tiles.
  - out projection K=512 (all 4 pairs) into PSUM, Pool copy, DMA to DRAM.
  Head loop interleaves window pairs (h_even,w0),(h_odd,w0),(h_even,w1),
  (h_odd,w1) per round; projections for pair p+1 and V for the next two
  heads are PE filler inside round p's exp windows.
"""

import itertools
import sys

if "/opt/trn_rl_repo" not in sys.path:
    sys.path.insert(0, "/opt/trn_rl_repo")

import numpy as np

import concourse.bass as bass
import concourse.tile as tile
from concourse import bacc
from concourse import mybir
from concourse import bass_utils

B, S, D, H, DH = 4, 2048, 1024, 16, 64
HL = 8              # heads per core
NCORES = 8
F32 = mybir.dt.float32
BF16 = mybir.dt.bfloat16
EXP = mybir.ActivationFunctionType.Exp

NDC = D // 128      # 8 d-chunks of 128
NKC = S // 128      # 16 s_k chunks of 128
NW = 2              # sq windows of 1024
NC8 = 8             # 128-wide sq chunks per window


def _kernel_body(tc):
    nc = tc.nc
    xT = nc.dram_tensor("xT", (D, S), F32, kind="ExternalInput").ap()
    wq = nc.dram_tensor("wq", (D, HL * DH), F32, kind="ExternalInput").ap()
    wk = nc.dram_tensor("wk", (D, HL * DH), F32, kind="ExternalInput").ap()
    wv = nc.dram_tensor("wv", (D, HL * DH), F32, kind="ExternalInput").ap()
    wo = nc.dram_tensor("wo", (HL * DH, D), F32, kind="ExternalInput").ap()
    out = nc.dram_tensor("out", (S, D), F32, kind="ExternalOutput").ap()

    cnt = itertools.count()
    conv_engines = itertools.cycle([nc.vector, nc.gpsimd])

    with tc.tile_pool(name="persist", bufs=1) as persist:
        qt = [persist.tile([128, S], BF16, name=f"qt{p}", tag=f"qt{p}") for p in range(4)]
        kt = [persist.tile([128, S], BF16, name=f"kt{p}", tag=f"kt{p}") for p in range(4)]
        vv = [persist.tile([128, HL * 65], BF16, name=f"v{sc}", tag=f"v{sc}") for sc in range(NKC)]
        ot = [persist.tile([128, S], BF16, name=f"ot{p}", tag=f"ot{p}") for p in range(4)]
        wqb = [persist.tile([128, HL * DH], BF16, name=f"wqb{dc}", tag=f"wqb{dc}") for dc in range(NDC)]
        wkb = [persist.tile([128, HL * DH], BF16, name=f"wkb{dc}", tag=f"wkb{dc}") for dc in range(NDC)]
        wvb = [persist.tile([128, HL * DH], BF16, name=f"wvb{dc}", tag=f"wvb{dc}") for dc in range(NDC)]

        with tc.tile_pool(name="stage", bufs=5) as stage, \
             tc.tile_pool(name="ptsp", bufs=26) as ptsp, \
             tc.tile_pool(name="onatp", bufs=18) as onatp, \
             tc.tile_pool(name="rrp", bufs=8) as rrp, \
             tc.tile_pool(name="stp", bufs=3) as stp, \
             tc.tile_pool(name="psS", bufs=2, space="PSUM") as psS, \
             tc.tile_pool(name="psO", bufs=2, space="PSUM") as psO, \
             tc.tile_pool(name="ppp", bufs=2, space="PSUM") as ppp:

            # bf16 x^T tiles live until the last projection (round-2 filler).
            # Pools release in LIFO order, so this one sits on top of the
            # stack and is swapped for the wo bf16 tiles at round 3.
            xbp = tc.alloc_tile_pool(name="xbp", bufs=1)
            xb = [xbp.tile([128, S], BF16, name=f"xb{dc}", tag=f"xb{dc}")
                  for dc in range(NDC)]

            wob = [None] * 4
            pts_map = {}
            onat_map = {}
            ppp_of = {"pp": ppp, "ps": psS}

            def load(dram_slice, dst_slice, shape, eng=None):
                n = next(cnt)
                t = stage.tile(list(shape), F32, name=f"sg{n}", tag="sg")
                nc.sync.dma_start(out=t, in_=dram_slice)
                (eng or next(conv_engines)).tensor_copy(dst_slice, t)

            def load_x_half(xh):
                csl = slice(xh * 1024, (xh + 1) * 1024)
                for dc in range(NDC):
                    load(xT[dc * 128:(dc + 1) * 128, csl], xb[dc][:, csl], (128, 1024))

            def load_w_cols(dram_w, dst_list, c0, c1):
                for dc in range(NDC):
                    load(dram_w[dc * 128:(dc + 1) * 128, c0:c1],
                         dst_list[dc][:, c0:c1], (128, c1 - c0))

            def qk_proj(which, p, s4):
                wsrc = wqb if which == "q" else wkb
                dst = (qt if which == "q" else kt)[p]
                ssl = slice(s4 * 512, (s4 + 1) * 512)
                ps = ppp.tile([128, 512], F32, name=f"pp_{which}{p}_{s4}", tag="pp")
                for dc in range(NDC):
                    nc.tensor.matmul(ps, wsrc[dc][:, p * 128:(p + 1) * 128],
                                     xb[dc][:, ssl], start=(dc == 0), stop=(dc == NDC - 1))
                nc.vector.tensor_copy(dst[:, ssl], ps)

            def v_proj(h, scs=None):
                for sc in (scs if scs is not None else range(NKC)):
                    ps = ppp.tile([128, DH], F32, name=f"pv_{h}_{sc}", tag="pp")
                    for dc in range(NDC):
                        nc.tensor.matmul(ps, xb[dc][:, sc * 128:(sc + 1) * 128],
                                         wvb[dc][:, h * DH:(h + 1) * DH],
                                         start=(dc == 0), stop=(dc == NDC - 1))
                    nc.vector.tensor_copy(vv[sc][:, h * 65:h * 65 + 64], ps)

            def scores(h, w, kcs):
                p, rh = h // 2, h % 2
                rsl = slice(rh * 64, rh * 64 + 64)
                dct = pts_map.setdefault((h, w), {})
                for kc in kcs:
                    ps = psS.tile([128, 1024], F32, name=f"ps_{h}_{w}_{kc}", tag="ps")
                    for half in range(2):
                        q0 = w * 1024 + half * 512
                        nc.tensor.matmul(ps[:, half * 512:(half + 1) * 512],
                                         kt[p][rsl, kc * 128:(kc + 1) * 128],
                                         qt[p][rsl, q0:q0 + 512],
                                         start=True, stop=True)
                    pt = ptsp.tile([128, 1024], BF16, name=f"pt_{h}_{w}_{kc}", tag="pt")
                    nc.scalar.activation(pt, ps, EXP, scale=0.125)
                    dct[kc] = pt

            def outproj_chunk(w, c, drain=False):
                # drain mode: ACT (idle after the last exp) does the PSUM->SBUF
                # copies, and psF alternates between the pp and ps PSUM rings
                # for a depth-4 pipeline so the PE never stalls on a copy.
                q16 = w * 8 + c
                qsl = slice(q16 * 128, (q16 + 1) * 128)
                for dcol in range(2):
                    tag = ("ps" if (drain and dcol == 1) else "pp")
                    shape = [128, 1024] if tag == "ps" else [128, 512]
                    pf = ppp_of[tag].tile(shape, F32,
                                          name=f"pf_{q16}_{dcol}", tag=tag)
                    pf = pf[:, 0:512]
                    for p in range(4):
                        nc.tensor.matmul(pf, ot[p][:, qsl],
                                         wob[p][:, dcol * 512:(dcol + 1) * 512],
                                         start=(p == 0), stop=(p == 3))
                    st = stp.tile([128, 512], F32,
                                  name=f"st_{q16}_{dcol}", tag="st")
                    if drain:
                        nc.scalar.copy(st, pf)
                    else:
                        nc.vector.tensor_copy(st, pf)
                    nc.sync.dma_start(out=out[qsl, dcol * 512:(dcol + 1) * 512],
                                      in_=st)

            def pe_transpose(p, w, c, on, eng):
                # XBAR DMA transpose RTT is ~3.5us through SP/HWDGE/DGE/sem;
                # a PE transpose (128 cycles) + engine copy lands in ~1us and
                # keeps the SP DMA queue free for the output writes.
                tp = psO.tile([128, 128], BF16, name=f"tp_{w}_{c}", tag="po")
                nc.tensor.transpose(tp, on, ident)
                dst = ot[p][:, w * 1024 + c * 128: w * 1024 + (c + 1) * 128]
                if eng is nc.scalar:
                    eng.copy(dst, tp)
                else:
                    eng.tensor_copy(dst, tp)

            def attn_v_chunk(h, w, c, P, transpose=True):
                p, rh = h // 2, h % 2
                po = psO.tile([128, 65], F32, name=f"po_{h}_{w}_{c}", tag="po")
                for kc in range(NKC):
                    nc.tensor.matmul(po, P[kc][:, c * 128:(c + 1) * 128],
                                     vv[kc][:, h * 65:(h + 1) * 65],
                                     start=(kc == 0), stop=(kc == NKC - 1))
                rr = rrp.tile([128, 1], F32, name=f"rr_{h}_{w}_{c}", tag="rr")
                nc.vector.reciprocal(rr, po[:, 64:65])
                if rh == 0:
                    on = onatp.tile([128, 128], BF16, name=f"on_{w}_{c}_{p}", tag="on")
                    onat_map[(w, c)] = on
                else:
                    on = onat_map.pop((w, c))
                nc.vector.tensor_mul(on[:, rh * 64:(rh + 1) * 64], po[:, 0:64],
                                     rr[:, 0:1].broadcast_to((128, 64)))
                if rh == 1 and transpose:
                    pe_transpose(p, w, c, on, nc.vector)
                return on

            def attn_v(h, w):
                P = pts_map.pop((h, w))
                for c in range(NC8):
                    attn_v_chunk(h, w, c, P)

            def attn_v_drain(h, w):
                P = pts_map.pop((h, w))
                for c in range(NC8):
                    attn_v_chunk(h, w, c, P)
                    if c >= 2:
                        outproj_chunk(w, c - 2, drain=True)
                for c in range(NC8 - 2, NC8):
                    outproj_chunk(w, c, drain=True)

            # ---------------- fill ----------------
            ident = persist.tile([128, 128], BF16, name="ident", tag="ident")
            onesq = persist.tile([128, 128], BF16, name="onesq", tag="onesq")
            nc.gpsimd.memset(onesq, 1.0)
            # iota(p, j) = p - j; keep where == 0 -> identity matrix
            nc.gpsimd.affine_select(ident, onesq, pattern=[[-1, 128]],
                                    compare_op=mybir.AluOpType.is_equal,
                                    fill=0.0, base=0, channel_multiplier=1)
            for sc in range(NKC):
                nc.gpsimd.memset(vv[sc], 1.0)
            load_x_half(0)
            load_w_cols(wq, wqb, 0, 128)   # pair 0
            load_w_cols(wk, wkb, 0, 128)
            load_x_half(1)
            load_w_cols(wv, wvb, 0, 512)

            # everything below "k2" needs the second x half; h0/h1 scores on
            # the first 8 kc chunks keep ACT fed until it lands.
            qk_proj("q", 0, 0)
            qk_proj("q", 0, 1)
            qk_proj("k", 0, 0)
            scores(0, 0, range(0, 4))
            qk_proj("k", 0, 1)
            scores(0, 0, range(4, 8))
            scores(1, 0, range(0, 8))
            qk_proj("k", 0, 2)
            qk_proj("k", 0, 3)
            scores(0, 0, range(8, 16))
            scores(1, 0, range(8, 16))
            qk_proj("q", 0, 2)
            qk_proj("q", 0, 3)
            v_proj(0)

            # PE filler (projections for later heads) per (round, window),
            # balanced so no window's filler exceeds the ~10us of PE slack
            # inside one 16.6us exp window, and so xb's last reader is in
            # round 2's second window (freeing its space for the wo tiles
            # well before the out-projection needs them).
            def F(*items):
                return list(items)
            Q, K, V = "q", "k", None
            fillers = {
                (0, 1): F(("v", 1), (Q, 1, 0), (Q, 1, 1)),
                (0, 2): F(("v", 2), (K, 1, 0), (K, 1, 1)),
                (0, 3): F(("v", 3), (Q, 1, 2), (Q, 1, 3), (K, 1, 2), (K, 1, 3)),
                (1, 1): F(("v", 4), (Q, 2, 0), (Q, 2, 1)),
                (1, 2): F(("v", 5), (K, 2, 0), (K, 2, 1)),
                (1, 3): F((K, 2, 2), (K, 2, 3), (Q, 2, 2), (Q, 2, 3),
                          (Q, 3, 0), (Q, 3, 1)),
                (2, 1): F(("v", 6), (Q, 3, 2), (Q, 3, 3), (K, 3, 0), (K, 3, 1)),
                (2, 2): F(("v", 7), (K, 3, 2), (K, 3, 3)),
            }

            def run_fillers(r, wslot):
                for it in fillers.get((r, wslot), []):
                    if it[0] == "v":
                        v_proj(it[1])
                    else:
                        qk_proj(it[0], it[1], it[2])

            # ---------------- rounds ----------------
            for r in range(4):
                a, b = 2 * r, 2 * r + 1
                if r == 0:
                    load_w_cols(wk, wkb, 128, 512)
                    load_w_cols(wq, wqb, 128, 512)
                if r == 3:
                    # xb space is free by now (last reader was round-2 W2
                    # filler); reuse it for the wo bf16 tiles. Conversions go
                    # on gpsimd so the DVE stream is not blocked behind them.
                    xbp.release()
                    wobp = tc.alloc_tile_pool(name="wobp", bufs=1)
                    for p in range(4):
                        wob[p] = wobp.tile([128, D], BF16, name=f"wob{p}", tag=f"wob{p}")
                        for half in range(2):
                            load(wo[p * 128:(p + 1) * 128, half * 512:(half + 1) * 512],
                                 wob[p][:, half * 512:(half + 1) * 512], (128, 512),
                                 eng=nc.gpsimd)
                if r > 0:
                    scores(a, 0, range(NKC))
                    attn_v(b - 2, 1)
                    scores(b, 0, range(NKC))
                attn_v(a, 0)
                run_fillers(r, 1)
                scores(a, 1, range(NKC))
                attn_v(b, 0)
                run_fillers(r, 2)
                scores(b, 1, range(NKC))
                if r == 3:
                    # outproj(w0) split around attn_v(6,1): the first half
                    # fills the PE while e(6,1) finishes; attn_v(6,1) then
                    # frees the P ring for e(7,1) before the second half.
                    for c in range(4):
                        outproj_chunk(0, c)
                attn_v(a, 1)
                if r == 3:
                    for c in range(4, NC8):
                        outproj_chunk(0, c)
                run_fillers(r, 3)

            # ---------------- drain ----------------
            attn_v_drain(7, 1)
            wobp.release()


_NC_CACHE = None


def _get_nc():
    global _NC_CACHE
    if _NC_CACHE is None:
        nc = bacc.Bacc("TRN2", target_bir_lowering=False, debug=False)
        with tile.TileContext(nc) as tc:
            _kernel_body(tc)
        nc.compile()
        _NC_CACHE = nc
    return _NC_CACHE


def _shard_inputs(x, Wq, Wk, Wv, Wo):
    in_maps = []
    for c in range(NCORES):
        b, g = c // 2, c % 2
        xT = np.ascontiguousarray(x[b].T)
        sl = slice(HL * g, HL * (g + 1))
        wq_s = np.ascontiguousarray(Wq[sl].transpose(1, 0, 2).reshape(D, HL * DH))
        wk_s = np.ascontiguousarray(Wk[sl].transpose(1, 0, 2).reshape(D, HL * DH))
        wv_s = np.ascontiguousarray(Wv[sl].transpose(1, 0, 2).reshape(D, HL * DH))
        wo_s = np.ascontiguousarray(Wo[HL * DH * g: HL * DH * (g + 1), :])
        in_maps.append({"xT": xT, "wq": wq_s, "wk": wk_s, "wv": wv_s, "wo": wo_s})
    return in_maps


def kernel(**inputs):
    x = np.asarray(inputs["x"], dtype=np.float32)
    Wq = np.asarray(inputs["Wq"], dtype=np.float32)
    Wk = np.asarray(inputs["Wk"], dtype=np.float32)
    Wv = np.asarray(inputs["Wv"], dtype=np.float32)
    Wo = np.asarray(inputs["Wo"], dtype=np.float32)

    nc = _get_nc()
    in_maps = _shard_inputs(x, Wq, Wk, Wv, Wo)
    res = None
    for attempt in range(3):
        try:
            res = bass_utils.run_bass_kernel_spmd(nc, in_maps, core_ids=list(range(NCORES)))
            break
        except Exception:
            # transient axon/NRT device errors recover on retry
            if attempt == 2:
                raise
            import time
            time.sleep(20)
    outs = [res.results[c]["out"] for c in range(NCORES)]
    full = np.stack([outs[2 * b] + outs[2 * b + 1] for b in range(B)], axis=0)
    return full.astype(np.float32)
